# revision 44
# baseline (speedup 1.0000x reference)
"""Trainium2 Bass kernel for nn_CrossAttention (tanh-scored, reversed-weight
attention), collective-free replicated-KV design.

Math (reference):
    q = x1 @ Wq.T + bq ; k = x2 @ Wk.T + bk ; v = x2 @ Wv.T + bv
    attn = softmax(tanh(q @ k.T) / sqrt(512), axis=-1)
    out  = ((1 - attn) / (N-1)) @ v

Kernel algebra (identical to the validated baseline):
    t_ij = tanh(q_i . k_j)                        (biases folded into q, k)
    e_ij = exp(scale * t_ij) ~= 1 + scale * t_ij  (|scale*t| <= 0.0442; the
          quadratic remainder cancels between softmax numerator/denominator)
    r_i  = N + scale * sum_j t_ij
    out_i = cv/(N-1) + bv - cv * rinv_i/(N-1) - (t^T@vraw)_i * scale*rinv_i/(N-1)
    with cv = colsum(vraw) = colsum(x2) @ Wv.T computed in fp32.

Sharding (per spec hint's replication option): rows of x_1 are sharded across
the 8 cores; x_2 and the weights are REPLICATED, so each core projects the
full K/V locally and no collective is needed.  Inputs are staged host-side in
the on-chip layout (feature dim on partitions, j-blocked for contiguous DMA):
x2 ships as bf16 and is cast on-device to fp8 for the PE projections; the
cast's accum_out simultaneously produces the f32-accumulated colsum(x2) for
the fp32 colsum-v path (that term dominates the output numerically; everything
flowing through the tanh scores is suppressed by 1/N and tolerates fp8).

All heavy matmuls (k/v/q projections, scores, attn@v) run fp8 DoubleRow.
PSUM: one pool of [128,2,512] double-bank tiles (bufs=2, 4 banks) shared by
projections / scores+tanh pairs / small epilogue outputs, plus 4 single-bank
attn-v accumulators = 8 banks exactly.  Rowsums are done on the PE after each
i-half's main loop from the persisted tanh tiles, accumulating into a spare
half of a pooled PSUM tile, so no dedicated rowsum bank is needed.

`build_kernel(repeat=R)` wraps the identical per-iteration body in a tc.For_i
hardware loop; test.py times two repeat counts and reports the slope, which
cancels the (machine-dependent, ~80 ms) per-dispatch RPC overhead of the axon
tunnel and yields the true steady-state HW time per kernel execution.
"""

import numpy as np
from contextlib import ExitStack

import concourse.bass as bass
import concourse.mybir as mybir
import concourse.tile as tile
from concourse import bacc
from concourse.bass_utils import run_bass_kernel_spmd

F32 = mybir.dt.float32
BF16 = mybir.dt.bfloat16
FP8 = mybir.dt.float8e4

NCORES = 8
N = 8192             # total rows (keys/values)
CIN = 1024           # input feature dim
D = 512              # d_kq = d_v
P = 128              # partitions
S = N // NCORES      # query rows per core (1024)
NCC = CIN // P       # 8 feature chunks
NDC = D // P         # 4 d chunks
NJB = 16             # x2 streaming blocks
JB = N // NJB        # 512 j columns per block
NJC = N // P         # 64 j chunks
SCALE = 1.0 / np.sqrt(np.float32(D))
INV_NM1 = 1.0 / np.float32(N - 1)
ACT_COPY = mybir.ActivationFunctionType.Copy
ACT_IDENT = mybir.ActivationFunctionType.Identity
ACT_TANH = mybir.ActivationFunctionType.Tanh
DR = mybir.MatmulPerfMode.DoubleRow


def emit_body(nc, tc, io, persist_tiles, pools):
    """Emit one full kernel iteration (projections + attention + epilogue)."""
    x1t, x2t, wqt, wkt, wvt, wvt32, bqt, bkt, bv, out = io
    ones2, ones_row = persist_tiles
    (wpool, kvpool, tfull, loads8, loadsbf, epool, cspool, ps2,
     ps_av_pool) = pools

    # ---- weight / bias / x1 loads; q-side first (gates the first PE work) ----
    wq_sb = wpool.tile([P, NCC, D], FP8, tag="wq")
    wk_sb = wpool.tile([P, NCC, D], FP8, tag="wk")
    wv_sb = wpool.tile([P, NCC, D], FP8, tag="wv")
    wv32_sb = wpool.tile([P, NCC, D], F32, tag="wv32")
    bq_sb = wpool.tile([P, NDC], F32, tag="bq")
    bk_sb = wpool.tile([P, NDC], F32, tag="bk")
    bv1 = wpool.tile([1, D], F32, tag="bv1")
    x1_sb = wpool.tile([P, NCC, S], FP8, tag="x1")
    nc.gpsimd.dma_start(out=wq_sb, in_=wqt[:, :, :])
    nc.gpsimd.dma_start(out=x1_sb, in_=x1t[:, :, :])
    nc.gpsimd.dma_start(out=bq_sb, in_=bqt[:, :])
    nc.gpsimd.dma_start(out=wk_sb, in_=wkt[:, :, :])
    nc.gpsimd.dma_start(out=wv_sb, in_=wvt[:, :, :])
    nc.gpsimd.dma_start(out=bk_sb, in_=bkt[:, :])
    nc.gpsimd.dma_start(out=bv1, in_=bv[:, :])

    # ---- q projection: qt[d, i] fp8, bias folded; [128,1024] drains ----
    qt = kvpool.tile([P, NDC, S], FP8, tag="qt")
    for di in range(NDC):
        pq = ps2.tile([P, 2, D], F32, tag="s2")
        for ih in range(2):
            for cp in range(NCC // 2):
                nc.tensor.matmul(
                    pq[:, ih, :],
                    lhsT=wq_sb[:, 2 * cp:2 * cp + 2, di * P:(di + 1) * P],
                    rhs=x1_sb[:, 2 * cp:2 * cp + 2, ih * D:(ih + 1) * D],
                    perf_mode=DR, start=(cp == 0), stop=(cp == NCC // 2 - 1))
        nc.scalar.activation(out=qt[:, di, :], in_=pq,
                             func=ACT_IDENT, bias=bq_sb[:, di:di + 1])

    # ---- streamed k/v projection over 16 j-blocks of 512 ----
    # Software-pipelined emission: DMAs run 3 blocks ahead and casts 1 block
    # ahead of the projection+drain work, so a drain waiting on the PE never
    # head-of-line-blocks the next block's casts in the in-order DVE/ACT
    # queues.
    kt = kvpool.tile([P, NDC, N], FP8, tag="kt")        # kT[d, j]
    vv = kvpool.tile([P, NJC, D], FP8, tag="v")         # v[j, d] (no bias)
    cs_part = cspool.tile([P, NCC, NJB], F32, tag="csp")

    def emit_dma(jb):
        x2b = loadsbf.tile([P, NCC, JB], BF16, tag="x2b", name=f"x2b_{jb}")
        # alternate queues so the 16 MB stream rides two DMA channels
        if jb % 2 == 0:
            nc.sync.dma_start(out=x2b, in_=x2t[jb, :, :, :])
        else:
            nc.gpsimd.dma_start(out=x2b, in_=x2t[jb, :, :, :])
        return x2b

    def emit_cast(jb, x2b):
        x2f = loads8.tile([P, NCC, JB], FP8, tag="x2f", name=f"x2f_{jb}")
        # fused bf16->fp8 cast + f32-accumulated colsum via accum_out
        for cc in range(NCC):
            if cc < 4:
                nc.vector.tensor_scalar(
                    x2f[:, cc, :], x2b[:, cc, :], 1.0, 0.0,
                    op0=mybir.AluOpType.mult, op1=mybir.AluOpType.add,
                    accum_out=cs_part[:, cc, jb:jb + 1])
            else:
                nc.scalar.activation(
                    out=x2f[:, cc, :], in_=x2b[:, cc, :], func=ACT_COPY,
                    accum_out=cs_part[:, cc, jb:jb + 1])
        return x2f

    x2bs = {jb: emit_dma(jb) for jb in range(min(3, NJB))}
    x2fs = {0: emit_cast(0, x2bs[0])}
    for jb in range(NJB):
        if jb + 3 < NJB:
            x2bs[jb + 3] = emit_dma(jb + 3)
        if jb + 1 < NJB:
            x2fs[jb + 1] = emit_cast(jb + 1, x2bs[jb + 1])
        x2f = x2fs[jb]
        # kT block: [512 d, 512 j]; di pairs share a double-bank psum tile,
        # drains stay [128,512] because the bias differs per di
        for dp in range(NDC // 2):
            pk = ps2.tile([P, 2, D], F32, tag="s2")
            for dh in range(2):
                di = 2 * dp + dh
                for cp in range(NCC // 2):
                    nc.tensor.matmul(
                        pk[:, dh, :],
                        lhsT=wk_sb[:, 2 * cp:2 * cp + 2, di * P:(di + 1) * P],
                        rhs=x2f[:, 2 * cp:2 * cp + 2, :],
                        perf_mode=DR, start=(cp == 0),
                        stop=(cp == NCC // 2 - 1))
                nc.vector.tensor_scalar_add(
                    kt[:, di, jb * JB:(jb + 1) * JB], pk[:, dh, :],
                    bk_sb[:, di:di + 1])
        # v block: [512 j, 512 dv]; jl pairs drained as one [128,2,512] op
        for vp in range(JB // P // 2):
            pv = ps2.tile([P, 2, D], F32, tag="s2")
            for vh in range(2):
                jl = 2 * vp + vh
                for cp in range(NCC // 2):
                    nc.tensor.matmul(
                        pv[:, vh, :],
                        lhsT=x2f[:, 2 * cp:2 * cp + 2, jl * P:(jl + 1) * P],
                        rhs=wv_sb[:, 2 * cp:2 * cp + 2, :],
                        perf_mode=DR, start=(cp == 0),
                        stop=(cp == NCC // 2 - 1))
            jj = jb * (JB // P) + 2 * vp
            nc.scalar.activation(out=vv[:, jj:jj + 2, :], in_=pv, func=ACT_COPY)

    # wv32 is only needed now (cv); its load rides after the stream dispatches
    nc.gpsimd.dma_start(out=wv32_sb, in_=wvt32[:, :, :])

    # ---- colsum_v (fp32) + broadcast helpers ----
    cs = cspool.tile([P, NCC], F32, tag="cs")
    nc.vector.reduce_sum(out=cs, in_=cs_part, axis=mybir.AxisListType.X)
    ps_cv = ps2.tile([P, 2, D], F32, tag="s2")
    for ci in range(NCC):
        nc.tensor.matmul(ps_cv[0:1, 0, :], lhsT=cs[:, ci:ci + 1],
                         rhs=wv32_sb[:, ci, :],
                         start=(ci == 0), stop=(ci == NCC - 1))
    cv1 = cspool.tile([1, D], F32, tag="cv1")
    nc.scalar.activation(out=cv1, in_=ps_cv[0:1, 0, :], func=ACT_COPY)
    cvd1 = cspool.tile([1, D], F32, tag="cvd1")
    nc.vector.tensor_scalar_mul(cvd1, cv1, float(INV_NM1))
    nc.vector.tensor_add(cvd1, cvd1, bv1)
    ps_b = ps2.tile([P, 2, D], F32, tag="s2")
    nc.tensor.matmul(ps_b[:, 0, :], lhsT=ones_row, rhs=cv1,
                     start=True, stop=True)
    nc.tensor.matmul(ps_b[:, 1, :], lhsT=ones_row, rhs=cvd1,
                     start=True, stop=True)
    cv_b = cspool.tile([P, D], F32, tag="cvb")
    nc.vector.tensor_copy(out=cv_b, in_=ps_b[:, 0, :])
    cvd_b = cspool.tile([P, D], F32, tag="cvdb")
    nc.vector.tensor_copy(out=cvd_b, in_=ps_b[:, 1, :])

    # ---- main attention loop, one i-half (512 rows) at a time ----
    for ih in range(2):
        ps_av = [ps2.tile([P, 2, D], F32, tag="s2", name=f"av{ap}_{ih}")
                 for ap in range(2)]
        t2f = tfull.tile([P, NJC, D], FP8, tag="t2f")
        for jp in range(NJC // 2):
            ps_s = ps2.tile([P, 2, D], F32, tag="s2")
            for sh in range(2):
                jc = 2 * jp + sh
                for qp in range(2):
                    nc.tensor.matmul(
                        ps_s[:, sh, :],
                        lhsT=kt[:, 2 * qp:2 * qp + 2, jc * P:(jc + 1) * P],
                        rhs=qt[:, 2 * qp:2 * qp + 2, ih * D:(ih + 1) * D],
                        perf_mode=DR, start=(qp == 0), stop=(qp == 1))
            nc.scalar.activation(out=t2f[:, 2 * jp:2 * jp + 2, :], in_=ps_s,
                                 func=ACT_TANH)
            first = (jp == 0)
            last = (jp == NJC // 2 - 1)
            for si in range(4):
                nc.tensor.matmul(
                    ps_av[si // 2][:, si % 2, :],
                    lhsT=t2f[:, 2 * jp:2 * jp + 2, si * P:(si + 1) * P],
                    rhs=vv[:, 2 * jp:2 * jp + 2, :],
                    perf_mode=DR, start=first, stop=last)

        # ---- rowsum from the persisted tanh tiles (PE, post-loop) ----
        ps_r = ps2.tile([P, 2, D], F32, tag="s2")
        for jc in range(NJC):
            nc.tensor.matmul(ps_r[0:1, 0, :], lhsT=ones2,
                             rhs=t2f[:, jc, :],
                             start=(jc == 0), stop=(jc == NJC - 1))

        # ---- epilogue for this i-half ----
        racc = epool.tile([1, D], F32, tag="racc")
        nc.scalar.activation(out=racc, in_=ps_r[0:1, 0, :], func=ACT_COPY)
        for si in range(4):
            nc.tensor.matmul(ps_r[:, 1, si:si + 1],
                             lhsT=racc[0:1, si * P:(si + 1) * P],
                             rhs=ones_row[0:1, 0:1], start=True, stop=True)
        rinv = epool.tile([P, 4], F32, tag="rinv")
        nc.vector.tensor_scalar(rinv, ps_r[:, 1, 0:4], float(SCALE), float(N),
                                op0=mybir.AluOpType.mult,
                                op1=mybir.AluOpType.add)
        nc.vector.reciprocal(rinv, rinv)
        ra = epool.tile([P, 4], F32, tag="ra")    # rinv/(N-1)
        nc.vector.tensor_scalar_mul(ra, rinv, float(INV_NM1))
        rb = epool.tile([P, 4], F32, tag="rb")    # rinv*scale/(N-1)
        nc.vector.tensor_scalar_mul(rb, rinv, float(SCALE * INV_NM1))
        for si in range(4):
            o1 = epool.tile([P, D], F32, tag="o1")
            nc.vector.tensor_scalar_mul(o1, ps_av[si // 2][:, si % 2, :],
                                        rb[:, si:si + 1])
            o2 = epool.tile([P, D], F32, tag="o2")
            nc.gpsimd.tensor_scalar_mul(o2, cv_b, ra[:, si:si + 1])
            nc.vector.tensor_sub(o1, cvd_b, o1)
            nc.vector.tensor_sub(o1, o1, o2)
            nc.sync.dma_start(
                out=out[ih * D + si * P: ih * D + (si + 1) * P, :], in_=o1)


def build_kernel(repeat: int = 1):
    nc = bacc.Bacc(num_devices=NCORES)

    x1t = nc.declare_dram_parameter("x1t", [P, NCC, S], FP8, isOutput=False)
    x2t = nc.declare_dram_parameter("x2t", [NJB, P, NCC, JB], BF16,
                                    isOutput=False)
    wqt = nc.declare_dram_parameter("wqt", [P, NCC, D], FP8, isOutput=False)
    wkt = nc.declare_dram_parameter("wkt", [P, NCC, D], FP8, isOutput=False)
    wvt = nc.declare_dram_parameter("wvt", [P, NCC, D], FP8, isOutput=False)
    wvt32 = nc.declare_dram_parameter("wvt32", [P, NCC, D], F32, isOutput=False)
    bqt = nc.declare_dram_parameter("bqt", [P, NDC], F32, isOutput=False)
    bkt = nc.declare_dram_parameter("bkt", [P, NDC], F32, isOutput=False)
    bv = nc.declare_dram_parameter("bv", [1, D], F32, isOutput=False)
    out = nc.declare_dram_parameter("out", [S, D], F32, isOutput=True)
    io = (x1t, x2t, wqt, wkt, wvt, wvt32, bqt, bkt, bv, out)

    with tile.TileContext(nc) as tc, ExitStack() as ctx:
        persist = ctx.enter_context(tc.tile_pool(name="persist", bufs=1))
        ones2 = persist.tile([P, 1], FP8)         # rowsum lhsT
        nc.vector.memset(ones2, 1.0)
        ones_row = persist.tile([1, P], F32)      # broadcast helper
        nc.vector.memset(ones_row, 1.0)
        persist_tiles = (ones2, ones_row)

        wpool = ctx.enter_context(tc.tile_pool(name="weights", bufs=1))
        kvpool = ctx.enter_context(tc.tile_pool(name="kv", bufs=1))
        tfull = ctx.enter_context(tc.tile_pool(name="tfull", bufs=1))
        loads8 = ctx.enter_context(tc.tile_pool(name="loads8", bufs=2))
        loadsbf = ctx.enter_context(tc.tile_pool(name="loadsbf", bufs=4))
        epool = ctx.enter_context(tc.tile_pool(name="epool", bufs=2))
        cspool = ctx.enter_context(tc.tile_pool(name="cspool", bufs=1))
        ps2 = ctx.enter_context(tc.tile_pool(name="ps2", bufs=4, space="PSUM"))
        pools = (wpool, kvpool, tfull, loads8, loadsbf, epool, cspool, ps2,
                 None)

        if repeat == 1:
            emit_body(nc, tc, io, persist_tiles, pools)
        else:
            with tc.For_i(0, repeat, 1,
                          hint_engines=(mybir.EngineType.PE,
                                        mybir.EngineType.Activation,
                                        mybir.EngineType.DVE,
                                        mybir.EngineType.SP,
                                        mybir.EngineType.Pool)):
                emit_body(nc, tc, io, persist_tiles, pools)

    if not nc.is_finalized():
        nc.finalize()
    return nc


_NC_CACHE = {}


def _get_nc(repeat: int = 1):
    if repeat not in _NC_CACHE:
        _NC_CACHE[repeat] = build_kernel(repeat)
    return _NC_CACHE[repeat]


def make_in_maps(x_1, x_2, Wq, bq, Wk, bk, Wv, bv):
    f8 = mybir.dt.np(FP8)
    bf = mybir.dt.np(BF16)

    def chunked_t(a, dtype):
        # [rows, cin] -> transposed, feature-chunked [128, cin//128, rows]
        a = np.asarray(a, np.float32)
        cin, rows = a.shape[1], a.shape[0]
        return np.ascontiguousarray(
            a.T.reshape(cin // P, P, rows).transpose(1, 0, 2)).astype(dtype)

    def blocked(a):
        # [128, 8, N] -> j-blocked [16, 128, 8, 512] (contiguous per block)
        return np.ascontiguousarray(
            a.reshape(P, NCC, NJB, JB).transpose(2, 0, 1, 3))

    x1t = chunked_t(x_1, f8)                      # [128, 8, 8192]
    shared = {
        "x2t": blocked(chunked_t(x_2, bf)),
        "wqt": chunked_t(np.asarray(Wq), f8),     # [128, 8, 512]
        "wkt": chunked_t(np.asarray(Wk), f8),
        "wvt": chunked_t(np.asarray(Wv), f8),
        "wvt32": chunked_t(np.asarray(Wv), np.float32),
        "bqt": np.ascontiguousarray(
            np.asarray(bq, np.float32).reshape(NDC, P).T),
        "bkt": np.ascontiguousarray(
            np.asarray(bk, np.float32).reshape(NDC, P).T),
        "bv": np.asarray(bv, np.float32).reshape(1, D).copy(),
    }
    return [
        {"x1t": np.ascontiguousarray(x1t[:, :, c * S:(c + 1) * S]), **shared}
        for c in range(NCORES)
    ]


def kernel(x_1, x_2, Wq, bq, Wk, bk, Wv, bv):
    nc = _get_nc(1)
    in_maps = make_in_maps(x_1, x_2, Wq, bq, Wk, bk, Wv, bv)
    res = run_bass_kernel_spmd(nc, in_maps, core_ids=list(range(NCORES)))
    return np.concatenate([res.results[c]["out"] for c in range(NCORES)], axis=0)


# revision 46
# speedup vs baseline: 1.0518x; 1.0518x over previous
"""Trainium2 Bass kernel for nn_CrossAttention (tanh-scored, reversed-weight
attention), collective-free replicated-KV design.

Math (reference):
    q = x1 @ Wq.T + bq ; k = x2 @ Wk.T + bk ; v = x2 @ Wv.T + bv
    attn = softmax(tanh(q @ k.T) / sqrt(512), axis=-1)
    out  = ((1 - attn) / (N-1)) @ v

Kernel algebra (identical to the validated baseline):
    t_ij = tanh(q_i . k_j)                        (biases folded into q, k)
    e_ij = exp(scale * t_ij) ~= 1 + scale * t_ij  (|scale*t| <= 0.0442; the
          quadratic remainder cancels between softmax numerator/denominator)
    r_i  = N + scale * sum_j t_ij
    out_i = cv/(N-1) + bv - cv * rinv_i/(N-1) - (t^T@vraw)_i * scale*rinv_i/(N-1)
    with cv = colsum(vraw) = colsum(x2) @ Wv.T computed in fp32.

Sharding (per spec hint's replication option): rows of x_1 are sharded across
the 8 cores; x_2 and the weights are REPLICATED, so each core projects the
full K/V locally and no collective is needed.  Inputs are staged host-side in
the on-chip layout (feature dim on partitions, j-blocked for contiguous DMA):
x2 ships as bf16 and is cast on-device to fp8 for the PE projections; the
cast's accum_out simultaneously produces the f32-accumulated colsum(x2) for
the fp32 colsum-v path (that term dominates the output numerically; everything
flowing through the tanh scores is suppressed by 1/N and tolerates fp8).

All heavy matmuls (k/v/q projections, scores, attn@v) run fp8 DoubleRow.
PSUM: one pool of [128,2,512] double-bank tiles (bufs=2, 4 banks) shared by
projections / scores+tanh pairs / small epilogue outputs, plus 4 single-bank
attn-v accumulators = 8 banks exactly.  Rowsums are done on the PE after each
i-half's main loop from the persisted tanh tiles, accumulating into a spare
half of a pooled PSUM tile, so no dedicated rowsum bank is needed.

`build_kernel(repeat=R)` wraps the identical per-iteration body in a tc.For_i
hardware loop; test.py times two repeat counts and reports the slope, which
cancels the (machine-dependent, ~80 ms) per-dispatch RPC overhead of the axon
tunnel and yields the true steady-state HW time per kernel execution.
"""

import numpy as np
from contextlib import ExitStack

import concourse.bass as bass
import concourse.mybir as mybir
import concourse.tile as tile
from concourse import bacc
from concourse.bass_utils import run_bass_kernel_spmd

F32 = mybir.dt.float32
BF16 = mybir.dt.bfloat16
FP8 = mybir.dt.float8e4

NCORES = 8
N = 8192             # total rows (keys/values)
CIN = 1024           # input feature dim
D = 512              # d_kq = d_v
P = 128              # partitions
S = N // NCORES      # query rows per core (1024)
NCC = CIN // P       # 8 feature chunks
NDC = D // P         # 4 d chunks
NJB = 16             # x2 streaming blocks
JB = N // NJB        # 512 j columns per block
NJC = N // P         # 64 j chunks
SCALE = 1.0 / np.sqrt(np.float32(D))
INV_NM1 = 1.0 / np.float32(N - 1)
ACT_COPY = mybir.ActivationFunctionType.Copy
ACT_IDENT = mybir.ActivationFunctionType.Identity
ACT_TANH = mybir.ActivationFunctionType.Tanh
DR = mybir.MatmulPerfMode.DoubleRow


def emit_body(nc, tc, io, persist_tiles, pools):
    """Emit one full kernel iteration (projections + attention + epilogue)."""
    x1t, x2t, wqt, wkt, wvt, wvt32, bqt, bkt, bv, out = io
    ones2, ones_row = persist_tiles
    (wpool, kvpool, tfull, loads8, loadsbf, epool, cspool, ps2,
     ps_av_pool) = pools

    # ---- weight / bias / x1 loads; q-side first (gates the first PE work) ----
    wq_sb = wpool.tile([P, NCC, D], FP8, tag="wq")
    wk_sb = wpool.tile([P, NCC, D], FP8, tag="wk")
    wv_sb = wpool.tile([P, NCC, D], FP8, tag="wv")
    wv32_sb = wpool.tile([P, NCC, D], F32, tag="wv32")
    bq_sb = wpool.tile([P, NDC], F32, tag="bq")
    bk_sb = wpool.tile([P, NDC], F32, tag="bk")
    bv1 = wpool.tile([1, D], F32, tag="bv1")
    x1_sb = wpool.tile([P, NCC, S], FP8, tag="x1")
    nc.gpsimd.dma_start(out=wq_sb, in_=wqt[:, :, :])
    nc.gpsimd.dma_start(out=x1_sb, in_=x1t[:, :, :])
    nc.gpsimd.dma_start(out=bq_sb, in_=bqt[:, :])
    nc.gpsimd.dma_start(out=wk_sb, in_=wkt[:, :, :])
    nc.gpsimd.dma_start(out=wv_sb, in_=wvt[:, :, :])
    nc.gpsimd.dma_start(out=bk_sb, in_=bkt[:, :])
    nc.gpsimd.dma_start(out=bv1, in_=bv[:, :])

    # ---- q projection: qt[d, i] fp8, bias folded; [128,1024] drains ----
    qt = kvpool.tile([P, NDC, S], FP8, tag="qt")
    for di in range(NDC):
        pq = ps2.tile([P, 2, D], F32, tag="s2")
        for ih in range(2):
            for cp in range(NCC // 2):
                nc.tensor.matmul(
                    pq[:, ih, :],
                    lhsT=wq_sb[:, 2 * cp:2 * cp + 2, di * P:(di + 1) * P],
                    rhs=x1_sb[:, 2 * cp:2 * cp + 2, ih * D:(ih + 1) * D],
                    perf_mode=DR, start=(cp == 0), stop=(cp == NCC // 2 - 1))
        nc.scalar.activation(out=qt[:, di, :], in_=pq,
                             func=ACT_IDENT, bias=bq_sb[:, di:di + 1])

    # ---- streamed k/v projection over 16 j-blocks of 512 ----
    kt = kvpool.tile([P, NDC, N], FP8, tag="kt")        # kT[d, j]
    vv = kvpool.tile([P, NJC, D], FP8, tag="v")         # v[j, d] (no bias)
    cs_part = cspool.tile([P, NCC, NJB], F32, tag="csp")
    for jb in range(NJB):
        x2b = loadsbf.tile([P, NCC, JB], BF16, tag="x2b")
        # alternate queues so the 16 MB stream rides two DMA channels
        if jb % 2 == 0:
            nc.sync.dma_start(out=x2b, in_=x2t[jb, :, :, :])
        else:
            nc.gpsimd.dma_start(out=x2b, in_=x2t[jb, :, :, :])
        x2f = loads8.tile([P, NCC, JB], FP8, tag="x2f")
        # fused bf16->fp8 cast + f32-accumulated colsum via accum_out
        for cc in range(NCC):
            if cc < 4:
                nc.vector.tensor_scalar(
                    x2f[:, cc, :], x2b[:, cc, :], 1.0, 0.0,
                    op0=mybir.AluOpType.mult, op1=mybir.AluOpType.add,
                    accum_out=cs_part[:, cc, jb:jb + 1])
            else:
                nc.scalar.activation(
                    out=x2f[:, cc, :], in_=x2b[:, cc, :], func=ACT_COPY,
                    accum_out=cs_part[:, cc, jb:jb + 1])
        # kT block: [512 d, 512 j]; di pairs share a double-bank psum tile,
        # drains stay [128,512] because the bias differs per di
        for dp in range(NDC // 2):
            pk = ps2.tile([P, 2, D], F32, tag="s2")
            for dh in range(2):
                di = 2 * dp + dh
                for cp in range(NCC // 2):
                    nc.tensor.matmul(
                        pk[:, dh, :],
                        lhsT=wk_sb[:, 2 * cp:2 * cp + 2, di * P:(di + 1) * P],
                        rhs=x2f[:, 2 * cp:2 * cp + 2, :],
                        perf_mode=DR, start=(cp == 0),
                        stop=(cp == NCC // 2 - 1))
                nc.vector.tensor_scalar_add(
                    kt[:, di, jb * JB:(jb + 1) * JB], pk[:, dh, :],
                    bk_sb[:, di:di + 1])
        # v block: [512 j, 512 dv]; jl pairs drained as one [128,2,512] op
        for vp in range(JB // P // 2):
            pv = ps2.tile([P, 2, D], F32, tag="s2")
            for vh in range(2):
                jl = 2 * vp + vh
                for cp in range(NCC // 2):
                    nc.tensor.matmul(
                        pv[:, vh, :],
                        lhsT=x2f[:, 2 * cp:2 * cp + 2, jl * P:(jl + 1) * P],
                        rhs=wv_sb[:, 2 * cp:2 * cp + 2, :],
                        perf_mode=DR, start=(cp == 0),
                        stop=(cp == NCC // 2 - 1))
            jj = jb * (JB // P) + 2 * vp
            nc.scalar.activation(out=vv[:, jj:jj + 2, :], in_=pv, func=ACT_COPY)

    # wv32 is only needed now (cv); its load rides after the stream dispatches
    nc.gpsimd.dma_start(out=wv32_sb, in_=wvt32[:, :, :])

    # ---- colsum_v (fp32) + broadcast helpers ----
    cs = cspool.tile([P, NCC], F32, tag="cs")
    nc.vector.reduce_sum(out=cs, in_=cs_part, axis=mybir.AxisListType.X)
    ps_cv = ps2.tile([P, 2, D], F32, tag="s2")
    for ci in range(NCC):
        nc.tensor.matmul(ps_cv[0:1, 0, :], lhsT=cs[:, ci:ci + 1],
                         rhs=wv32_sb[:, ci, :],
                         start=(ci == 0), stop=(ci == NCC - 1))
    cv1 = cspool.tile([1, D], F32, tag="cv1")
    nc.scalar.activation(out=cv1, in_=ps_cv[0:1, 0, :], func=ACT_COPY)
    cvd1 = cspool.tile([1, D], F32, tag="cvd1")
    nc.vector.tensor_scalar_mul(cvd1, cv1, float(INV_NM1))
    nc.vector.tensor_add(cvd1, cvd1, bv1)
    ps_b = ps2.tile([P, 2, D], F32, tag="s2")
    nc.tensor.matmul(ps_b[:, 0, :], lhsT=ones_row, rhs=cv1,
                     start=True, stop=True)
    nc.tensor.matmul(ps_b[:, 1, :], lhsT=ones_row, rhs=cvd1,
                     start=True, stop=True)
    cv_b = cspool.tile([P, D], F32, tag="cvb")
    nc.vector.tensor_copy(out=cv_b, in_=ps_b[:, 0, :])
    cvd_b = cspool.tile([P, D], F32, tag="cvdb")
    nc.vector.tensor_copy(out=cvd_b, in_=ps_b[:, 1, :])

    # ---- main attention loop, one i-half (512 rows) at a time ----
    for ih in range(2):
        ps_av = [ps2.tile([P, 2, D], F32, tag="s2", name=f"av{ap}_{ih}")
                 for ap in range(2)]
        t2f = tfull.tile([P, NJC, D], FP8, tag="t2f")
        for jp in range(NJC // 2):
            ps_s = ps2.tile([P, 2, D], F32, tag="s2")
            for sh in range(2):
                jc = 2 * jp + sh
                for qp in range(2):
                    nc.tensor.matmul(
                        ps_s[:, sh, :],
                        lhsT=kt[:, 2 * qp:2 * qp + 2, jc * P:(jc + 1) * P],
                        rhs=qt[:, 2 * qp:2 * qp + 2, ih * D:(ih + 1) * D],
                        perf_mode=DR, start=(qp == 0), stop=(qp == 1))
            nc.scalar.activation(out=t2f[:, 2 * jp:2 * jp + 2, :], in_=ps_s,
                                 func=ACT_TANH)
            first = (jp == 0)
            last = (jp == NJC // 2 - 1)
            for si in range(4):
                nc.tensor.matmul(
                    ps_av[si // 2][:, si % 2, :],
                    lhsT=t2f[:, 2 * jp:2 * jp + 2, si * P:(si + 1) * P],
                    rhs=vv[:, 2 * jp:2 * jp + 2, :],
                    perf_mode=DR, start=first, stop=last)

        # ---- rowsum from the persisted tanh tiles (PE, post-loop) ----
        ps_r = ps2.tile([P, 2, D], F32, tag="s2")
        for jc in range(NJC):
            nc.tensor.matmul(ps_r[0:1, 0, :], lhsT=ones2,
                             rhs=t2f[:, jc, :],
                             start=(jc == 0), stop=(jc == NJC - 1))

        # ---- epilogue for this i-half ----
        racc = epool.tile([1, D], F32, tag="racc")
        nc.scalar.activation(out=racc, in_=ps_r[0:1, 0, :], func=ACT_COPY)
        for si in range(4):
            nc.tensor.matmul(ps_r[:, 1, si:si + 1],
                             lhsT=racc[0:1, si * P:(si + 1) * P],
                             rhs=ones_row[0:1, 0:1], start=True, stop=True)
        rinv = epool.tile([P, 4], F32, tag="rinv")
        nc.vector.tensor_scalar(rinv, ps_r[:, 1, 0:4], float(SCALE), float(N),
                                op0=mybir.AluOpType.mult,
                                op1=mybir.AluOpType.add)
        nc.vector.reciprocal(rinv, rinv)
        ra = epool.tile([P, 4], F32, tag="ra")    # rinv/(N-1)
        nc.vector.tensor_scalar_mul(ra, rinv, float(INV_NM1))
        rb = epool.tile([P, 4], F32, tag="rb")    # rinv*scale/(N-1)
        nc.vector.tensor_scalar_mul(rb, rinv, float(SCALE * INV_NM1))
        for si in range(4):
            o1 = epool.tile([P, D], F32, tag="o1")
            nc.vector.tensor_scalar_mul(o1, ps_av[si // 2][:, si % 2, :],
                                        rb[:, si:si + 1])
            o2 = epool.tile([P, D], F32, tag="o2")
            nc.gpsimd.tensor_scalar_mul(o2, cv_b, ra[:, si:si + 1])
            nc.vector.tensor_sub(o1, cvd_b, o1)
            nc.vector.tensor_sub(o1, o1, o2)
            nc.sync.dma_start(
                out=out[ih * D + si * P: ih * D + (si + 1) * P, :], in_=o1)


def build_kernel(repeat: int = 1):
    nc = bacc.Bacc(num_devices=NCORES)

    x1t = nc.declare_dram_parameter("x1t", [P, NCC, S], FP8, isOutput=False)
    x2t = nc.declare_dram_parameter("x2t", [NJB, P, NCC, JB], BF16,
                                    isOutput=False)
    wqt = nc.declare_dram_parameter("wqt", [P, NCC, D], FP8, isOutput=False)
    wkt = nc.declare_dram_parameter("wkt", [P, NCC, D], FP8, isOutput=False)
    wvt = nc.declare_dram_parameter("wvt", [P, NCC, D], FP8, isOutput=False)
    wvt32 = nc.declare_dram_parameter("wvt32", [P, NCC, D], F32, isOutput=False)
    bqt = nc.declare_dram_parameter("bqt", [P, NDC], F32, isOutput=False)
    bkt = nc.declare_dram_parameter("bkt", [P, NDC], F32, isOutput=False)
    bv = nc.declare_dram_parameter("bv", [1, D], F32, isOutput=False)
    out = nc.declare_dram_parameter("out", [S, D], F32, isOutput=True)
    io = (x1t, x2t, wqt, wkt, wvt, wvt32, bqt, bkt, bv, out)

    with tile.TileContext(nc) as tc, ExitStack() as ctx:
        persist = ctx.enter_context(tc.tile_pool(name="persist", bufs=1))
        ones2 = persist.tile([P, 1], FP8)         # rowsum lhsT
        nc.vector.memset(ones2, 1.0)
        ones_row = persist.tile([1, P], F32)      # broadcast helper
        nc.vector.memset(ones_row, 1.0)
        persist_tiles = (ones2, ones_row)

        wpool = ctx.enter_context(tc.tile_pool(name="weights", bufs=1))
        kvpool = ctx.enter_context(tc.tile_pool(name="kv", bufs=1))
        tfull = ctx.enter_context(tc.tile_pool(name="tfull", bufs=1))
        loads8 = ctx.enter_context(tc.tile_pool(name="loads8", bufs=3))
        loadsbf = ctx.enter_context(tc.tile_pool(name="loadsbf", bufs=3))
        epool = ctx.enter_context(tc.tile_pool(name="epool", bufs=2))
        cspool = ctx.enter_context(tc.tile_pool(name="cspool", bufs=1))
        ps2 = ctx.enter_context(tc.tile_pool(name="ps2", bufs=4, space="PSUM"))
        pools = (wpool, kvpool, tfull, loads8, loadsbf, epool, cspool, ps2,
                 None)

        if repeat == 1:
            emit_body(nc, tc, io, persist_tiles, pools)
        else:
            with tc.For_i(0, repeat, 1,
                          hint_engines=(mybir.EngineType.PE,
                                        mybir.EngineType.Activation,
                                        mybir.EngineType.DVE,
                                        mybir.EngineType.SP,
                                        mybir.EngineType.Pool)):
                emit_body(nc, tc, io, persist_tiles, pools)

    if not nc.is_finalized():
        nc.finalize()
    return nc


_NC_CACHE = {}


def _get_nc(repeat: int = 1):
    if repeat not in _NC_CACHE:
        _NC_CACHE[repeat] = build_kernel(repeat)
    return _NC_CACHE[repeat]


def make_in_maps(x_1, x_2, Wq, bq, Wk, bk, Wv, bv):
    f8 = mybir.dt.np(FP8)
    bf = mybir.dt.np(BF16)

    def chunked_t(a, dtype):
        # [rows, cin] -> transposed, feature-chunked [128, cin//128, rows]
        a = np.asarray(a, np.float32)
        cin, rows = a.shape[1], a.shape[0]
        return np.ascontiguousarray(
            a.T.reshape(cin // P, P, rows).transpose(1, 0, 2)).astype(dtype)

    def blocked(a):
        # [128, 8, N] -> j-blocked [16, 128, 8, 512] (contiguous per block)
        return np.ascontiguousarray(
            a.reshape(P, NCC, NJB, JB).transpose(2, 0, 1, 3))

    x1t = chunked_t(x_1, f8)                      # [128, 8, 8192]
    shared = {
        "x2t": blocked(chunked_t(x_2, bf)),
        "wqt": chunked_t(np.asarray(Wq), f8),     # [128, 8, 512]
        "wkt": chunked_t(np.asarray(Wk), f8),
        "wvt": chunked_t(np.asarray(Wv), f8),
        "wvt32": chunked_t(np.asarray(Wv), np.float32),
        "bqt": np.ascontiguousarray(
            np.asarray(bq, np.float32).reshape(NDC, P).T),
        "bkt": np.ascontiguousarray(
            np.asarray(bk, np.float32).reshape(NDC, P).T),
        "bv": np.asarray(bv, np.float32).reshape(1, D).copy(),
    }
    return [
        {"x1t": np.ascontiguousarray(x1t[:, :, c * S:(c + 1) * S]), **shared}
        for c in range(NCORES)
    ]


def kernel(x_1, x_2, Wq, bq, Wk, bk, Wv, bv):
    nc = _get_nc(1)
    in_maps = make_in_maps(x_1, x_2, Wq, bq, Wk, bk, Wv, bv)
    res = run_bass_kernel_spmd(nc, in_maps, core_ids=list(range(NCORES)))
    return np.concatenate([res.results[c]["out"] for c in range(NCORES)], axis=0)


# revision 50
# speedup vs baseline: 1.1525x; 1.0958x over previous
"""Trainium2 Bass kernel for nn_CrossAttention (tanh-scored, reversed-weight
attention), collective-free replicated-KV design.

Math (reference):
    q = x1 @ Wq.T + bq ; k = x2 @ Wk.T + bk ; v = x2 @ Wv.T + bv
    attn = softmax(tanh(q @ k.T) / sqrt(512), axis=-1)
    out  = ((1 - attn) / (N-1)) @ v

Kernel algebra (identical to the validated baseline):
    t_ij = tanh(q_i . k_j)                        (biases folded into q, k)
    e_ij = exp(scale * t_ij) ~= 1 + scale * t_ij  (|scale*t| <= 0.0442; the
          quadratic remainder cancels between softmax numerator/denominator)
    r_i  = N + scale * sum_j t_ij
    out_i = cv/(N-1) + bv - cv * rinv_i/(N-1) - (t^T@vraw)_i * scale*rinv_i/(N-1)
    with cv = colsum(vraw) = colsum(x2) @ Wv.T computed in fp32.

Sharding (per spec hint's replication option): rows of x_1 are sharded across
the 8 cores; x_2 and the weights are REPLICATED, so each core projects the
full K/V locally and no collective is needed.  Inputs are staged host-side in
the on-chip layout (feature dim on partitions, j-blocked for contiguous DMA):
x2 ships as bf16 and is cast on-device to fp8 for the PE projections; the
cast's accum_out simultaneously produces the f32-accumulated colsum(x2) for
the fp32 colsum-v path (that term dominates the output numerically; everything
flowing through the tanh scores is suppressed by 1/N and tolerates fp8).

All heavy matmuls (k/v/q projections, scores, attn@v) run fp8 DoubleRow.
PSUM: one pool of [128,2,512] double-bank tiles (bufs=2, 4 banks) shared by
projections / scores+tanh pairs / small epilogue outputs, plus 4 single-bank
attn-v accumulators = 8 banks exactly.  Rowsums are done on the PE after each
i-half's main loop from the persisted tanh tiles, accumulating into a spare
half of a pooled PSUM tile, so no dedicated rowsum bank is needed.

`build_kernel(repeat=R)` wraps the identical per-iteration body in a tc.For_i
hardware loop; test.py times two repeat counts and reports the slope, which
cancels the (machine-dependent, ~80 ms) per-dispatch RPC overhead of the axon
tunnel and yields the true steady-state HW time per kernel execution.
"""

import numpy as np
from contextlib import ExitStack

import concourse.bass as bass
import concourse.mybir as mybir
import concourse.tile as tile
from concourse import bacc
from concourse.bass_utils import run_bass_kernel_spmd

F32 = mybir.dt.float32
BF16 = mybir.dt.bfloat16
FP8 = mybir.dt.float8e4

NCORES = 8
N = 8192             # total rows (keys/values)
CIN = 1024           # input feature dim
D = 512              # d_kq = d_v
P = 128              # partitions
S = N // NCORES      # query rows per core (1024)
NCC = CIN // P       # 8 feature chunks
NDC = D // P         # 4 d chunks
NJB = 16             # x2 streaming blocks
JB = N // NJB        # 512 j columns per block
NJC = N // P         # 64 j chunks
SCALE = 1.0 / np.sqrt(np.float32(D))
INV_NM1 = 1.0 / np.float32(N - 1)
ACT_COPY = mybir.ActivationFunctionType.Copy
ACT_IDENT = mybir.ActivationFunctionType.Identity
ACT_TANH = mybir.ActivationFunctionType.Tanh
DR = mybir.MatmulPerfMode.DoubleRow


def emit_body(nc, tc, io, persist_tiles, pools):
    """Emit one full kernel iteration (projections + attention + epilogue)."""
    x1t, x2t, wqt, wkt, wvt, wvt32, bqt, bkt, bv, out = io
    (ones_row,) = persist_tiles
    (wpool, kvpool, tfull, loads8, loadsbf, epool, cspool, ps2,
     ps_av_pool) = pools

    # ---- weight / bias / x1 loads; q-side first (gates the first PE work) ----
    wq_sb = wpool.tile([P, NCC, D], FP8, tag="wq")
    wk_sb = wpool.tile([P, NCC, D], FP8, tag="wk")
    wv_sb = wpool.tile([P, NCC, D], FP8, tag="wv")
    wv32_sb = wpool.tile([P, NCC, D], F32, tag="wv32")
    bq_sb = wpool.tile([P, NDC], F32, tag="bq")
    bk_sb = wpool.tile([P, NDC], F32, tag="bk")
    bv1 = wpool.tile([1, D], F32, tag="bv1")
    x1_sb = wpool.tile([P, NCC, S], FP8, tag="x1")
    nc.gpsimd.dma_start(out=wq_sb, in_=wqt[:, :, :])
    nc.gpsimd.dma_start(out=x1_sb, in_=x1t[:, :, :])
    nc.gpsimd.dma_start(out=bq_sb, in_=bqt[:, :])
    nc.gpsimd.dma_start(out=wk_sb, in_=wkt[:, :, :])
    nc.gpsimd.dma_start(out=wv_sb, in_=wvt[:, :, :])
    nc.gpsimd.dma_start(out=bk_sb, in_=bkt[:, :])
    nc.gpsimd.dma_start(out=bv1, in_=bv[:, :])

    # ---- q projection: qt[d, i] fp8, bias folded; [128,1024] drains ----
    qt = kvpool.tile([P, NDC, S], FP8, tag="qt")
    for di in range(NDC):
        pq = ps2.tile([P, 2, D], F32, tag="s2")
        for ih in range(2):
            for cp in range(NCC // 2):
                nc.tensor.matmul(
                    pq[:, ih, :],
                    lhsT=wq_sb[:, 2 * cp:2 * cp + 2, di * P:(di + 1) * P],
                    rhs=x1_sb[:, 2 * cp:2 * cp + 2, ih * D:(ih + 1) * D],
                    perf_mode=DR, start=(cp == 0), stop=(cp == NCC // 2 - 1))
        nc.scalar.activation(out=qt[:, di, :], in_=pq,
                             func=ACT_IDENT, bias=bq_sb[:, di:di + 1])

    # ---- streamed k/v projection over 16 j-blocks of 512 ----
    kt = kvpool.tile([P, NDC, N], FP8, tag="kt")        # kT[d, j]
    vv = kvpool.tile([P, NJC, D], FP8, tag="v")         # v[j, d] (no bias)
    # v column 511 is a ones-column: attn@v then lands the per-i rowsum of t
    # in accumulator column 511, exactly where the epilogue needs it.  The
    # dropped t@v contribution of v's true 512th column is ~2e-8 of the
    # output (the whole t@v correction is suppressed by scale/(N*(N-1))).
    nc.vector.memset(vv[:, :, D - 1:D], 1.0)
    cs_part = cspool.tile([P, NCC, NJB], F32, tag="csp")
    for jb in range(NJB):
        x2b = loadsbf.tile([P, NCC, JB], BF16, tag="x2b")
        # alternate queues so the 16 MB stream rides two DMA channels
        if jb % 2 == 0:
            nc.sync.dma_start(out=x2b, in_=x2t[jb, :, :, :])
        else:
            nc.gpsimd.dma_start(out=x2b, in_=x2t[jb, :, :, :])
        x2f = loads8.tile([P, NCC, JB], FP8, tag="x2f")
        # fused bf16->fp8 cast + f32-accumulated colsum via accum_out
        for cc in range(NCC):
            if cc < 4:
                nc.vector.tensor_scalar(
                    x2f[:, cc, :], x2b[:, cc, :], 1.0, 0.0,
                    op0=mybir.AluOpType.mult, op1=mybir.AluOpType.add,
                    accum_out=cs_part[:, cc, jb:jb + 1])
            else:
                nc.scalar.activation(
                    out=x2f[:, cc, :], in_=x2b[:, cc, :], func=ACT_COPY,
                    accum_out=cs_part[:, cc, jb:jb + 1])
        # kT block: [512 d, 512 j]; di pairs share a double-bank psum tile,
        # drains stay [128,512] because the bias differs per di
        for dp in range(NDC // 2):
            pk = ps2.tile([P, 2, D], F32, tag="s2")
            for dh in range(2):
                di = 2 * dp + dh
                for cp in range(NCC // 2):
                    nc.tensor.matmul(
                        pk[:, dh, :],
                        lhsT=wk_sb[:, 2 * cp:2 * cp + 2, di * P:(di + 1) * P],
                        rhs=x2f[:, 2 * cp:2 * cp + 2, :],
                        perf_mode=DR, start=(cp == 0),
                        stop=(cp == NCC // 2 - 1))
                nc.vector.tensor_scalar_add(
                    kt[:, di, jb * JB:(jb + 1) * JB], pk[:, dh, :],
                    bk_sb[:, di:di + 1])
        # v block: [512 j, 512 dv]; jl pairs drained as one [128,2,512] op
        for vp in range(JB // P // 2):
            pv = ps2.tile([P, 2, D], F32, tag="s2")
            for vh in range(2):
                jl = 2 * vp + vh
                for cp in range(NCC // 2):
                    nc.tensor.matmul(
                        pv[:, vh, :],
                        lhsT=x2f[:, 2 * cp:2 * cp + 2, jl * P:(jl + 1) * P],
                        rhs=wv_sb[:, 2 * cp:2 * cp + 2, :],
                        perf_mode=DR, start=(cp == 0),
                        stop=(cp == NCC // 2 - 1))
            jj = jb * (JB // P) + 2 * vp
            nc.scalar.activation(out=vv[:, jj:jj + 2, 0:D - 1],
                                 in_=pv[:, :, 0:D - 1], func=ACT_COPY)

    # wv32 is only needed now (cv); its load rides after the stream dispatches
    nc.gpsimd.dma_start(out=wv32_sb, in_=wvt32[:, :, :])

    # ---- colsum_v (fp32) + broadcast helpers ----
    cs = cspool.tile([P, NCC], F32, tag="cs")
    nc.vector.reduce_sum(out=cs, in_=cs_part, axis=mybir.AxisListType.X)
    ps_cv = ps2.tile([P, 2, D], F32, tag="s2")
    for ci in range(NCC):
        nc.tensor.matmul(ps_cv[0:1, 0, :], lhsT=cs[:, ci:ci + 1],
                         rhs=wv32_sb[:, ci, :],
                         start=(ci == 0), stop=(ci == NCC - 1))
    cv1 = cspool.tile([1, D], F32, tag="cv1")
    nc.scalar.activation(out=cv1, in_=ps_cv[0:1, 0, :], func=ACT_COPY)
    cvd1 = cspool.tile([1, D], F32, tag="cvd1")
    nc.vector.tensor_scalar_mul(cvd1, cv1, float(INV_NM1))
    nc.vector.tensor_add(cvd1, cvd1, bv1)
    ps_b = ps2.tile([P, 2, D], F32, tag="s2")
    nc.tensor.matmul(ps_b[:, 0, :], lhsT=ones_row, rhs=cv1,
                     start=True, stop=True)
    nc.tensor.matmul(ps_b[:, 1, :], lhsT=ones_row, rhs=cvd1,
                     start=True, stop=True)
    cv_b = cspool.tile([P, D], F32, tag="cvb")
    nc.vector.tensor_copy(out=cv_b, in_=ps_b[:, 0, :])
    cvd_b = cspool.tile([P, D], F32, tag="cvdb")
    nc.vector.tensor_copy(out=cvd_b, in_=ps_b[:, 1, :])

    # ---- main attention loop, one i-half (512 rows) at a time ----
    for ih in range(2):
        ps_av = [ps2.tile([P, 2, D], F32, tag="s2", name=f"av{ap}_{ih}")
                 for ap in range(2)]
        t2f = tfull.tile([P, NJC, D], FP8, tag="t2f")
        for jp in range(NJC // 2):
            ps_s = ps2.tile([P, 2, D], F32, tag="s2")
            for sh in range(2):
                jc = 2 * jp + sh
                for qp in range(2):
                    nc.tensor.matmul(
                        ps_s[:, sh, :],
                        lhsT=kt[:, 2 * qp:2 * qp + 2, jc * P:(jc + 1) * P],
                        rhs=qt[:, 2 * qp:2 * qp + 2, ih * D:(ih + 1) * D],
                        perf_mode=DR, start=(qp == 0), stop=(qp == 1))
            nc.scalar.activation(out=t2f[:, 2 * jp:2 * jp + 2, :], in_=ps_s,
                                 func=ACT_TANH)
            first = (jp == 0)
            last = (jp == NJC // 2 - 1)
            for si in range(4):
                nc.tensor.matmul(
                    ps_av[si // 2][:, si % 2, :],
                    lhsT=t2f[:, 2 * jp:2 * jp + 2, si * P:(si + 1) * P],
                    rhs=vv[:, 2 * jp:2 * jp + 2, :],
                    perf_mode=DR, start=first, stop=last)

        # ---- epilogue for this i-half: rowsum_i sits at accumulator col 511
        for si in range(4):
            avs = ps_av[si // 2][:, si % 2, :]
            rinv = epool.tile([P, 1], F32, tag="rinv")
            nc.vector.tensor_scalar(rinv, avs[:, D - 1:D], float(SCALE),
                                    float(N), op0=mybir.AluOpType.mult,
                                    op1=mybir.AluOpType.add)
            nc.vector.reciprocal(rinv, rinv)
            ra = epool.tile([P, 1], F32, tag="ra")    # rinv/(N-1)
            nc.vector.tensor_scalar_mul(ra, rinv, float(INV_NM1))
            rb = epool.tile([P, 1], F32, tag="rb")    # rinv*scale/(N-1)
            nc.vector.tensor_scalar_mul(rb, rinv, float(SCALE * INV_NM1))
            o1 = epool.tile([P, D], F32, tag="o1")
            nc.vector.tensor_scalar_mul(o1, avs, rb)
            o2 = epool.tile([P, D], F32, tag="o2")
            nc.gpsimd.tensor_scalar_mul(o2, cv_b, ra)
            nc.vector.tensor_sub(o1, cvd_b, o1)
            nc.vector.tensor_sub(o1, o1, o2)
            nc.sync.dma_start(
                out=out[ih * D + si * P: ih * D + (si + 1) * P, :], in_=o1)


def build_kernel(repeat: int = 1):
    nc = bacc.Bacc(num_devices=NCORES)

    x1t = nc.declare_dram_parameter("x1t", [P, NCC, S], FP8, isOutput=False)
    x2t = nc.declare_dram_parameter("x2t", [NJB, P, NCC, JB], BF16,
                                    isOutput=False)
    wqt = nc.declare_dram_parameter("wqt", [P, NCC, D], FP8, isOutput=False)
    wkt = nc.declare_dram_parameter("wkt", [P, NCC, D], FP8, isOutput=False)
    wvt = nc.declare_dram_parameter("wvt", [P, NCC, D], FP8, isOutput=False)
    wvt32 = nc.declare_dram_parameter("wvt32", [P, NCC, D], F32, isOutput=False)
    bqt = nc.declare_dram_parameter("bqt", [P, NDC], F32, isOutput=False)
    bkt = nc.declare_dram_parameter("bkt", [P, NDC], F32, isOutput=False)
    bv = nc.declare_dram_parameter("bv", [1, D], F32, isOutput=False)
    out = nc.declare_dram_parameter("out", [S, D], F32, isOutput=True)
    io = (x1t, x2t, wqt, wkt, wvt, wvt32, bqt, bkt, bv, out)

    with tile.TileContext(nc) as tc, ExitStack() as ctx:
        persist = ctx.enter_context(tc.tile_pool(name="persist", bufs=1))
        ones_row = persist.tile([1, P], F32)      # broadcast helper
        nc.vector.memset(ones_row, 1.0)
        persist_tiles = (ones_row,)

        wpool = ctx.enter_context(tc.tile_pool(name="weights", bufs=1))
        kvpool = ctx.enter_context(tc.tile_pool(name="kv", bufs=1))
        tfull = ctx.enter_context(tc.tile_pool(name="tfull", bufs=1))
        loads8 = ctx.enter_context(tc.tile_pool(name="loads8", bufs=3))
        loadsbf = ctx.enter_context(tc.tile_pool(name="loadsbf", bufs=3))
        epool = ctx.enter_context(tc.tile_pool(name="epool", bufs=2))
        cspool = ctx.enter_context(tc.tile_pool(name="cspool", bufs=1))
        ps2 = ctx.enter_context(tc.tile_pool(name="ps2", bufs=4, space="PSUM"))
        pools = (wpool, kvpool, tfull, loads8, loadsbf, epool, cspool, ps2,
                 None)

        if repeat == 1:
            emit_body(nc, tc, io, persist_tiles, pools)
        else:
            with tc.For_i(0, repeat, 1,
                          hint_engines=(mybir.EngineType.PE,
                                        mybir.EngineType.Activation,
                                        mybir.EngineType.DVE,
                                        mybir.EngineType.SP,
                                        mybir.EngineType.Pool)):
                emit_body(nc, tc, io, persist_tiles, pools)

    if not nc.is_finalized():
        nc.finalize()
    return nc


_NC_CACHE = {}


def _get_nc(repeat: int = 1):
    if repeat not in _NC_CACHE:
        _NC_CACHE[repeat] = build_kernel(repeat)
    return _NC_CACHE[repeat]


def make_in_maps(x_1, x_2, Wq, bq, Wk, bk, Wv, bv):
    f8 = mybir.dt.np(FP8)
    bf = mybir.dt.np(BF16)

    def chunked_t(a, dtype):
        # [rows, cin] -> transposed, feature-chunked [128, cin//128, rows]
        a = np.asarray(a, np.float32)
        cin, rows = a.shape[1], a.shape[0]
        return np.ascontiguousarray(
            a.T.reshape(cin // P, P, rows).transpose(1, 0, 2)).astype(dtype)

    def blocked(a):
        # [128, 8, N] -> j-blocked [16, 128, 8, 512] (contiguous per block)
        return np.ascontiguousarray(
            a.reshape(P, NCC, NJB, JB).transpose(2, 0, 1, 3))

    x1t = chunked_t(x_1, f8)                      # [128, 8, 8192]
    shared = {
        "x2t": blocked(chunked_t(x_2, bf)),
        "wqt": chunked_t(np.asarray(Wq), f8),     # [128, 8, 512]
        "wkt": chunked_t(np.asarray(Wk), f8),
        "wvt": chunked_t(np.asarray(Wv), f8),
        "wvt32": chunked_t(np.asarray(Wv), np.float32),
        "bqt": np.ascontiguousarray(
            np.asarray(bq, np.float32).reshape(NDC, P).T),
        "bkt": np.ascontiguousarray(
            np.asarray(bk, np.float32).reshape(NDC, P).T),
        "bv": np.asarray(bv, np.float32).reshape(1, D).copy(),
    }
    return [
        {"x1t": np.ascontiguousarray(x1t[:, :, c * S:(c + 1) * S]), **shared}
        for c in range(NCORES)
    ]


def kernel(x_1, x_2, Wq, bq, Wk, bk, Wv, bv):
    nc = _get_nc(1)
    in_maps = make_in_maps(x_1, x_2, Wq, bq, Wk, bk, Wv, bv)
    res = run_bass_kernel_spmd(nc, in_maps, core_ids=list(range(NCORES)))
    return np.concatenate([res.results[c]["out"] for c in range(NCORES)], axis=0)


# revision 52
# speedup vs baseline: 1.1541x; 1.0013x over previous
"""Trainium2 Bass kernel for nn_CrossAttention (tanh-scored, reversed-weight
attention), collective-free replicated-KV design.

Math (reference):
    q = x1 @ Wq.T + bq ; k = x2 @ Wk.T + bk ; v = x2 @ Wv.T + bv
    attn = softmax(tanh(q @ k.T) / sqrt(512), axis=-1)
    out  = ((1 - attn) / (N-1)) @ v

Kernel algebra (identical to the validated baseline):
    t_ij = tanh(q_i . k_j)                        (biases folded into q, k)
    e_ij = exp(scale * t_ij) ~= 1 + scale * t_ij  (|scale*t| <= 0.0442; the
          quadratic remainder cancels between softmax numerator/denominator)
    r_i  = N + scale * sum_j t_ij
    out_i = cv/(N-1) + bv - cv * rinv_i/(N-1) - (t^T@vraw)_i * scale*rinv_i/(N-1)
    with cv = colsum(vraw) = colsum(x2) @ Wv.T computed in fp32.

Sharding (per spec hint's replication option): rows of x_1 are sharded across
the 8 cores; x_2 and the weights are REPLICATED, so each core projects the
full K/V locally and no collective is needed.  Inputs are staged host-side in
the on-chip layout (feature dim on partitions, j-blocked for contiguous DMA):
x2 ships as bf16 and is cast on-device to fp8 for the PE projections; the
cast's accum_out simultaneously produces the f32-accumulated colsum(x2) for
the fp32 colsum-v path (that term dominates the output numerically; everything
flowing through the tanh scores is suppressed by 1/N and tolerates fp8).

All heavy matmuls (k/v/q projections, scores, attn@v) run fp8 DoubleRow.
PSUM: one pool of [128,2,512] double-bank tiles (bufs=4 = 8 banks) shared by
projections, scores+tanh pairs, and the attn-v accumulators (two double-bank
tiles per i-half).  v's last column is replaced by a ones-column, so the
attn-v accumulation lands the per-i rowsum of t directly in accumulator
column 511 in the exact per-partition layout the epilogue needs -- no rowsum
matmuls, no PE transpose, and no dedicated rowsum PSUM bank.  The dropped
t@v contribution of v's true 512th column is ~2e-8 of the output.

`build_kernel(repeat=R)` wraps the identical per-iteration body in a tc.For_i
hardware loop; test.py times two repeat counts and reports the slope, which
cancels the (machine-dependent, ~80 ms) per-dispatch RPC overhead of the axon
tunnel and yields the true steady-state HW time per kernel execution.
"""

import numpy as np
from contextlib import ExitStack

import concourse.bass as bass
import concourse.mybir as mybir
import concourse.tile as tile
from concourse import bacc
from concourse.bass_utils import run_bass_kernel_spmd

F32 = mybir.dt.float32
BF16 = mybir.dt.bfloat16
FP8 = mybir.dt.float8e4

NCORES = 8
N = 8192             # total rows (keys/values)
CIN = 1024           # input feature dim
D = 512              # d_kq = d_v
P = 128              # partitions
S = N // NCORES      # query rows per core (1024)
NCC = CIN // P       # 8 feature chunks
NDC = D // P         # 4 d chunks
NJB = 16             # x2 streaming blocks
JB = N // NJB        # 512 j columns per block
NJC = N // P         # 64 j chunks
SCALE = 1.0 / np.sqrt(np.float32(D))
INV_NM1 = 1.0 / np.float32(N - 1)
ACT_COPY = mybir.ActivationFunctionType.Copy
ACT_IDENT = mybir.ActivationFunctionType.Identity
ACT_TANH = mybir.ActivationFunctionType.Tanh
DR = mybir.MatmulPerfMode.DoubleRow


def emit_body(nc, tc, io, persist_tiles, pools):
    """Emit one full kernel iteration (projections + attention + epilogue)."""
    x1t, x2t, wqt, wkt, wvt, wvt32, bqt, bkt, bv, out = io
    (ones_row,) = persist_tiles
    (wpool, kvpool, tfull, loads8, loadsbf, epool, cspool, ps2,
     ps_av_pool) = pools

    # ---- weight / bias / x1 loads; q-side first (gates the first PE work) ----
    wq_sb = wpool.tile([P, NCC, D], FP8, tag="wq")
    wk_sb = wpool.tile([P, NCC, D], FP8, tag="wk")
    wv_sb = wpool.tile([P, NCC, D], FP8, tag="wv")
    wv32_sb = wpool.tile([P, NCC, D], F32, tag="wv32")
    bq_sb = wpool.tile([P, NDC], F32, tag="bq")
    bk_sb = wpool.tile([P, NDC], F32, tag="bk")
    bv1 = wpool.tile([1, D], F32, tag="bv1")
    x1_sb = wpool.tile([P, NCC, S], FP8, tag="x1")
    nc.gpsimd.dma_start(out=wq_sb, in_=wqt[:, :, :])
    nc.gpsimd.dma_start(out=x1_sb, in_=x1t[:, :, :])
    nc.gpsimd.dma_start(out=bq_sb, in_=bqt[:, :])
    nc.gpsimd.dma_start(out=wk_sb, in_=wkt[:, :, :])
    nc.gpsimd.dma_start(out=wv_sb, in_=wvt[:, :, :])
    nc.gpsimd.dma_start(out=bk_sb, in_=bkt[:, :])
    nc.gpsimd.dma_start(out=bv1, in_=bv[:, :])

    # ---- q projection: qt[d, i] fp8, bias folded; [128,1024] drains ----
    qt = kvpool.tile([P, NDC, S], FP8, tag="qt")
    for di in range(NDC):
        pq = ps2.tile([P, 2, D], F32, tag="s2")
        for ih in range(2):
            for cp in range(NCC // 2):
                nc.tensor.matmul(
                    pq[:, ih, :],
                    lhsT=wq_sb[:, 2 * cp:2 * cp + 2, di * P:(di + 1) * P],
                    rhs=x1_sb[:, 2 * cp:2 * cp + 2, ih * D:(ih + 1) * D],
                    perf_mode=DR, start=(cp == 0), stop=(cp == NCC // 2 - 1))
        nc.scalar.activation(out=qt[:, di, :], in_=pq,
                             func=ACT_IDENT, bias=bq_sb[:, di:di + 1])

    # ---- streamed k/v projection over 16 j-blocks of 512 ----
    kt = kvpool.tile([P, NDC, N], FP8, tag="kt")        # kT[d, j]
    vv = kvpool.tile([P, NJC, D], FP8, tag="v")         # v[j, d] (no bias)
    # v column 511 is a ones-column: attn@v then lands the per-i rowsum of t
    # in accumulator column 511, exactly where the epilogue needs it.  The
    # dropped t@v contribution of v's true 512th column is ~2e-8 of the
    # output (the whole t@v correction is suppressed by scale/(N*(N-1))).
    nc.vector.memset(vv[:, :, D - 1:D], 1.0)
    cs_part = cspool.tile([P, NCC, NJB], F32, tag="csp")
    for jb in range(NJB):
        x2b = loadsbf.tile([P, NCC, JB], BF16, tag="x2b")
        # alternate queues so the 16 MB stream rides two DMA channels
        if jb % 2 == 0:
            nc.sync.dma_start(out=x2b, in_=x2t[jb, :, :, :])
        else:
            nc.gpsimd.dma_start(out=x2b, in_=x2t[jb, :, :, :])
        x2f = loads8.tile([P, NCC, JB], FP8, tag="x2f")
        # fused bf16->fp8 cast + f32-accumulated colsum via accum_out
        for cc in range(NCC):
            if cc < 4:
                nc.vector.tensor_scalar(
                    x2f[:, cc, :], x2b[:, cc, :], 1.0, 0.0,
                    op0=mybir.AluOpType.mult, op1=mybir.AluOpType.add,
                    accum_out=cs_part[:, cc, jb:jb + 1])
            else:
                nc.scalar.activation(
                    out=x2f[:, cc, :], in_=x2b[:, cc, :], func=ACT_COPY,
                    accum_out=cs_part[:, cc, jb:jb + 1])
        # kT block: [512 d, 512 j]; di pairs share a double-bank psum tile,
        # drains stay [128,512] because the bias differs per di
        for dp in range(NDC // 2):
            pk = ps2.tile([P, 2, D], F32, tag="s2")
            for dh in range(2):
                di = 2 * dp + dh
                for cp in range(NCC // 2):
                    nc.tensor.matmul(
                        pk[:, dh, :],
                        lhsT=wk_sb[:, 2 * cp:2 * cp + 2, di * P:(di + 1) * P],
                        rhs=x2f[:, 2 * cp:2 * cp + 2, :],
                        perf_mode=DR, start=(cp == 0),
                        stop=(cp == NCC // 2 - 1))
                nc.vector.tensor_scalar_add(
                    kt[:, di, jb * JB:(jb + 1) * JB], pk[:, dh, :],
                    bk_sb[:, di:di + 1])
        # v block: [512 j, 512 dv]; jl pairs drained as one [128,2,512] op
        for vp in range(JB // P // 2):
            pv = ps2.tile([P, 2, D], F32, tag="s2")
            for vh in range(2):
                jl = 2 * vp + vh
                for cp in range(NCC // 2):
                    nc.tensor.matmul(
                        pv[:, vh, :],
                        lhsT=x2f[:, 2 * cp:2 * cp + 2, jl * P:(jl + 1) * P],
                        rhs=wv_sb[:, 2 * cp:2 * cp + 2, :],
                        perf_mode=DR, start=(cp == 0),
                        stop=(cp == NCC // 2 - 1))
            jj = jb * (JB // P) + 2 * vp
            nc.scalar.activation(out=vv[:, jj:jj + 2, 0:D - 1],
                                 in_=pv[:, :, 0:D - 1], func=ACT_COPY)

    # wv32 is only needed now (cv); its load rides after the stream dispatches
    nc.gpsimd.dma_start(out=wv32_sb, in_=wvt32[:, :, :])

    # ---- colsum_v (fp32) + broadcast helpers ----
    cs = cspool.tile([P, NCC], F32, tag="cs")
    nc.vector.reduce_sum(out=cs, in_=cs_part, axis=mybir.AxisListType.X)
    ps_cv = ps2.tile([P, 2, D], F32, tag="s2")
    for ci in range(NCC):
        nc.tensor.matmul(ps_cv[0:1, 0, :], lhsT=cs[:, ci:ci + 1],
                         rhs=wv32_sb[:, ci, :],
                         start=(ci == 0), stop=(ci == NCC - 1))
    cv1 = cspool.tile([1, D], F32, tag="cv1")
    nc.scalar.activation(out=cv1, in_=ps_cv[0:1, 0, :], func=ACT_COPY)
    cvd1 = cspool.tile([1, D], F32, tag="cvd1")
    nc.vector.tensor_scalar_mul(cvd1, cv1, float(INV_NM1))
    nc.vector.tensor_add(cvd1, cvd1, bv1)
    ps_b = ps2.tile([P, 2, D], F32, tag="s2")
    nc.tensor.matmul(ps_b[:, 0, :], lhsT=ones_row, rhs=cv1,
                     start=True, stop=True)
    nc.tensor.matmul(ps_b[:, 1, :], lhsT=ones_row, rhs=cvd1,
                     start=True, stop=True)
    cv_b = cspool.tile([P, D], F32, tag="cvb")
    nc.vector.tensor_copy(out=cv_b, in_=ps_b[:, 0, :])
    cvd_b = cspool.tile([P, D], F32, tag="cvdb")
    nc.vector.tensor_copy(out=cvd_b, in_=ps_b[:, 1, :])

    # ---- main attention loop, one i-half (512 rows) at a time ----
    for ih in range(2):
        ps_av = [ps2.tile([P, 2, D], F32, tag="s2", name=f"av{ap}_{ih}")
                 for ap in range(2)]
        t2f = tfull.tile([P, NJC, D], FP8, tag="t2f")
        for jp in range(NJC // 2):
            ps_s = ps2.tile([P, 2, D], F32, tag="s2")
            for sh in range(2):
                jc = 2 * jp + sh
                for qp in range(2):
                    nc.tensor.matmul(
                        ps_s[:, sh, :],
                        lhsT=kt[:, 2 * qp:2 * qp + 2, jc * P:(jc + 1) * P],
                        rhs=qt[:, 2 * qp:2 * qp + 2, ih * D:(ih + 1) * D],
                        perf_mode=DR, start=(qp == 0), stop=(qp == 1))
            nc.scalar.activation(out=t2f[:, 2 * jp:2 * jp + 2, :], in_=ps_s,
                                 func=ACT_TANH)
            first = (jp == 0)
            last = (jp == NJC // 2 - 1)
            for si in range(4):
                nc.tensor.matmul(
                    ps_av[si // 2][:, si % 2, :],
                    lhsT=t2f[:, 2 * jp:2 * jp + 2, si * P:(si + 1) * P],
                    rhs=vv[:, 2 * jp:2 * jp + 2, :],
                    perf_mode=DR, start=first, stop=last)

        # ---- epilogue for this i-half: rowsum_i sits at accumulator col 511
        for si in range(4):
            avs = ps_av[si // 2][:, si % 2, :]
            rinv = epool.tile([P, 1], F32, tag="rinv")
            nc.vector.tensor_scalar(rinv, avs[:, D - 1:D], float(SCALE),
                                    float(N), op0=mybir.AluOpType.mult,
                                    op1=mybir.AluOpType.add)
            nc.vector.reciprocal(rinv, rinv)
            ra = epool.tile([P, 1], F32, tag="ra")    # rinv/(N-1)
            nc.vector.tensor_scalar_mul(ra, rinv, float(INV_NM1))
            rb = epool.tile([P, 1], F32, tag="rb")    # rinv*scale/(N-1)
            nc.vector.tensor_scalar_mul(rb, rinv, float(SCALE * INV_NM1))
            o1 = epool.tile([P, D], F32, tag="o1")
            nc.vector.tensor_scalar_mul(o1, avs, rb)
            o2 = epool.tile([P, D], F32, tag="o2")
            nc.gpsimd.tensor_scalar_mul(o2, cv_b, ra)
            nc.vector.tensor_sub(o1, cvd_b, o1)
            nc.vector.tensor_sub(o1, o1, o2)
            nc.sync.dma_start(
                out=out[ih * D + si * P: ih * D + (si + 1) * P, :], in_=o1)


def build_kernel(repeat: int = 1):
    nc = bacc.Bacc(num_devices=NCORES)

    x1t = nc.declare_dram_parameter("x1t", [P, NCC, S], FP8, isOutput=False)
    x2t = nc.declare_dram_parameter("x2t", [NJB, P, NCC, JB], BF16,
                                    isOutput=False)
    wqt = nc.declare_dram_parameter("wqt", [P, NCC, D], FP8, isOutput=False)
    wkt = nc.declare_dram_parameter("wkt", [P, NCC, D], FP8, isOutput=False)
    wvt = nc.declare_dram_parameter("wvt", [P, NCC, D], FP8, isOutput=False)
    wvt32 = nc.declare_dram_parameter("wvt32", [P, NCC, D], F32, isOutput=False)
    bqt = nc.declare_dram_parameter("bqt", [P, NDC], F32, isOutput=False)
    bkt = nc.declare_dram_parameter("bkt", [P, NDC], F32, isOutput=False)
    bv = nc.declare_dram_parameter("bv", [1, D], F32, isOutput=False)
    out = nc.declare_dram_parameter("out", [S, D], F32, isOutput=True)
    io = (x1t, x2t, wqt, wkt, wvt, wvt32, bqt, bkt, bv, out)

    with tile.TileContext(nc) as tc, ExitStack() as ctx:
        persist = ctx.enter_context(tc.tile_pool(name="persist", bufs=1))
        ones_row = persist.tile([1, P], F32)      # broadcast helper
        nc.vector.memset(ones_row, 1.0)
        persist_tiles = (ones_row,)

        wpool = ctx.enter_context(tc.tile_pool(name="weights", bufs=1))
        kvpool = ctx.enter_context(tc.tile_pool(name="kv", bufs=1))
        tfull = ctx.enter_context(tc.tile_pool(name="tfull", bufs=1))
        loads8 = ctx.enter_context(tc.tile_pool(name="loads8", bufs=4))
        loadsbf = ctx.enter_context(tc.tile_pool(name="loadsbf", bufs=4))
        epool = ctx.enter_context(tc.tile_pool(name="epool", bufs=2))
        cspool = ctx.enter_context(tc.tile_pool(name="cspool", bufs=1))
        ps2 = ctx.enter_context(tc.tile_pool(name="ps2", bufs=4, space="PSUM"))
        pools = (wpool, kvpool, tfull, loads8, loadsbf, epool, cspool, ps2,
                 None)

        if repeat == 1:
            emit_body(nc, tc, io, persist_tiles, pools)
        else:
            with tc.For_i(0, repeat, 1,
                          hint_engines=(mybir.EngineType.PE,
                                        mybir.EngineType.Activation,
                                        mybir.EngineType.DVE,
                                        mybir.EngineType.SP,
                                        mybir.EngineType.Pool)):
                emit_body(nc, tc, io, persist_tiles, pools)

    if not nc.is_finalized():
        nc.finalize()
    return nc


_NC_CACHE = {}


def _get_nc(repeat: int = 1):
    if repeat not in _NC_CACHE:
        _NC_CACHE[repeat] = build_kernel(repeat)
    return _NC_CACHE[repeat]


def make_in_maps(x_1, x_2, Wq, bq, Wk, bk, Wv, bv):
    f8 = mybir.dt.np(FP8)
    bf = mybir.dt.np(BF16)

    def chunked_t(a, dtype):
        # [rows, cin] -> transposed, feature-chunked [128, cin//128, rows]
        a = np.asarray(a, np.float32)
        cin, rows = a.shape[1], a.shape[0]
        return np.ascontiguousarray(
            a.T.reshape(cin // P, P, rows).transpose(1, 0, 2)).astype(dtype)

    def blocked(a):
        # [128, 8, N] -> j-blocked [16, 128, 8, 512] (contiguous per block)
        return np.ascontiguousarray(
            a.reshape(P, NCC, NJB, JB).transpose(2, 0, 1, 3))

    x1t = chunked_t(x_1, f8)                      # [128, 8, 8192]
    shared = {
        "x2t": blocked(chunked_t(x_2, bf)),
        "wqt": chunked_t(np.asarray(Wq), f8),     # [128, 8, 512]
        "wkt": chunked_t(np.asarray(Wk), f8),
        "wvt": chunked_t(np.asarray(Wv), f8),
        "wvt32": chunked_t(np.asarray(Wv), np.float32),
        "bqt": np.ascontiguousarray(
            np.asarray(bq, np.float32).reshape(NDC, P).T),
        "bkt": np.ascontiguousarray(
            np.asarray(bk, np.float32).reshape(NDC, P).T),
        "bv": np.asarray(bv, np.float32).reshape(1, D).copy(),
    }
    return [
        {"x1t": np.ascontiguousarray(x1t[:, :, c * S:(c + 1) * S]), **shared}
        for c in range(NCORES)
    ]


def kernel(x_1, x_2, Wq, bq, Wk, bk, Wv, bv):
    nc = _get_nc(1)
    in_maps = make_in_maps(x_1, x_2, Wq, bq, Wk, bk, Wv, bv)
    res = run_bass_kernel_spmd(nc, in_maps, core_ids=list(range(NCORES)))
    return np.concatenate([res.results[c]["out"] for c in range(NCORES)], axis=0)


# revision 53
# speedup vs baseline: 1.1603x; 1.0054x over previous
"""Trainium2 Bass kernel for nn_CrossAttention (tanh-scored, reversed-weight
attention), collective-free replicated-KV design.

Math (reference):
    q = x1 @ Wq.T + bq ; k = x2 @ Wk.T + bk ; v = x2 @ Wv.T + bv
    attn = softmax(tanh(q @ k.T) / sqrt(512), axis=-1)
    out  = ((1 - attn) / (N-1)) @ v

Kernel algebra (identical to the validated baseline):
    t_ij = tanh(q_i . k_j)                        (biases folded into q, k)
    e_ij = exp(scale * t_ij) ~= 1 + scale * t_ij  (|scale*t| <= 0.0442; the
          quadratic remainder cancels between softmax numerator/denominator)
    r_i  = N + scale * sum_j t_ij
    out_i = cv/(N-1) + bv - cv * rinv_i/(N-1) - (t^T@vraw)_i * scale*rinv_i/(N-1)
    with cv = colsum(vraw) = colsum(x2) @ Wv.T computed in fp32.

Sharding (per spec hint's replication option): rows of x_1 are sharded across
the 8 cores; x_2 and the weights are REPLICATED, so each core projects the
full K/V locally and no collective is needed.  Inputs are staged host-side in
the on-chip layout (feature dim on partitions, j-blocked for contiguous DMA):
x2 ships as bf16 and is cast on-device to fp8 for the PE projections; the
cast's accum_out simultaneously produces the f32-accumulated colsum(x2) for
the fp32 colsum-v path (that term dominates the output numerically; everything
flowing through the tanh scores is suppressed by 1/N and tolerates fp8).

All heavy matmuls (k/v/q projections, scores, attn@v) run fp8 DoubleRow.
PSUM: one pool of [128,2,512] double-bank tiles (bufs=4 = 8 banks) shared by
projections, scores+tanh pairs, and the attn-v accumulators (two double-bank
tiles per i-half).  v's last column is replaced by a ones-column, so the
attn-v accumulation lands the per-i rowsum of t directly in accumulator
column 511 in the exact per-partition layout the epilogue needs -- no rowsum
matmuls, no PE transpose, and no dedicated rowsum PSUM bank.  The dropped
t@v contribution of v's true 512th column is ~2e-8 of the output.

`build_kernel(repeat=R)` wraps the identical per-iteration body in a tc.For_i
hardware loop; test.py times two repeat counts and reports the slope, which
cancels the (machine-dependent, ~80 ms) per-dispatch RPC overhead of the axon
tunnel and yields the true steady-state HW time per kernel execution.
"""

import numpy as np
from contextlib import ExitStack

import concourse.bass as bass
import concourse.mybir as mybir
import concourse.tile as tile
from concourse import bacc
from concourse.bass_utils import run_bass_kernel_spmd

F32 = mybir.dt.float32
BF16 = mybir.dt.bfloat16
FP8 = mybir.dt.float8e4

NCORES = 8
N = 8192             # total rows (keys/values)
CIN = 1024           # input feature dim
D = 512              # d_kq = d_v
P = 128              # partitions
S = N // NCORES      # query rows per core (1024)
NCC = CIN // P       # 8 feature chunks
NDC = D // P         # 4 d chunks
NJB = 16             # x2 streaming blocks
JB = N // NJB        # 512 j columns per block
NJC = N // P         # 64 j chunks
SCALE = 1.0 / np.sqrt(np.float32(D))
INV_NM1 = 1.0 / np.float32(N - 1)
ACT_COPY = mybir.ActivationFunctionType.Copy
ACT_IDENT = mybir.ActivationFunctionType.Identity
ACT_TANH = mybir.ActivationFunctionType.Tanh
DR = mybir.MatmulPerfMode.DoubleRow


def emit_body(nc, tc, io, persist_tiles, pools):
    """Emit one full kernel iteration (projections + attention + epilogue)."""
    x1t, x2t, wqt, wkt, wvt, wvt32, bqt, bkt, bv, out = io
    (ones_row,) = persist_tiles
    (wpool, kvpool, tfull, loads8, loadsbf, epool, cspool, ps2,
     ps_av_pool) = pools

    # ---- weight / bias / x1 loads; q-side first (gates the first PE work) ----
    wq_sb = wpool.tile([P, NCC, D], FP8, tag="wq")
    wk_sb = wpool.tile([P, NCC, D], FP8, tag="wk")
    wv_sb = wpool.tile([P, NCC, D], FP8, tag="wv")
    wv32_sb = wpool.tile([P, NCC, D], F32, tag="wv32")
    bq_sb = wpool.tile([P, NDC], F32, tag="bq")
    bk_sb = wpool.tile([P, NDC], F32, tag="bk")
    bv1 = wpool.tile([1, D], F32, tag="bv1")
    x1_sb = wpool.tile([P, NCC, S], FP8, tag="x1")
    nc.gpsimd.dma_start(out=wq_sb, in_=wqt[:, :, :])
    nc.gpsimd.dma_start(out=x1_sb, in_=x1t[:, :, :])
    nc.gpsimd.dma_start(out=bq_sb, in_=bqt[:, :])
    nc.gpsimd.dma_start(out=wk_sb, in_=wkt[:, :, :])
    nc.gpsimd.dma_start(out=wv_sb, in_=wvt[:, :, :])
    nc.gpsimd.dma_start(out=bk_sb, in_=bkt[:, :])
    nc.gpsimd.dma_start(out=bv1, in_=bv[:, :])

    # ---- q projection: qt[d, i] fp8, bias folded; [128,1024] drains ----
    qt = kvpool.tile([P, NDC, S], FP8, tag="qt")
    for di in range(NDC):
        pq = ps2.tile([P, 2, D], F32, tag="s2")
        for ih in range(2):
            for cp in range(NCC // 2):
                nc.tensor.matmul(
                    pq[:, ih, :],
                    lhsT=wq_sb[:, 2 * cp:2 * cp + 2, di * P:(di + 1) * P],
                    rhs=x1_sb[:, 2 * cp:2 * cp + 2, ih * D:(ih + 1) * D],
                    perf_mode=DR, start=(cp == 0), stop=(cp == NCC // 2 - 1))
        nc.scalar.activation(out=qt[:, di, :], in_=pq,
                             func=ACT_IDENT, bias=bq_sb[:, di:di + 1])

    # ---- streamed k/v projection over 16 j-blocks of 512 ----
    kt = kvpool.tile([P, NDC, N], FP8, tag="kt")        # kT[d, j]
    vv = kvpool.tile([P, NJC, D], FP8, tag="v")         # v[j, d] (no bias)
    # v column 511 is a ones-column: attn@v then lands the per-i rowsum of t
    # in accumulator column 511, exactly where the epilogue needs it.  The
    # dropped t@v contribution of v's true 512th column is ~2e-8 of the
    # output (the whole t@v correction is suppressed by scale/(N*(N-1))).
    nc.vector.memset(vv[:, :, D - 1:D], 1.0)
    cs_part = cspool.tile([P, NCC, NJB], F32, tag="csp")
    for jb in range(NJB):
        x2b = loadsbf.tile([P, NCC, JB], BF16, tag="x2b")
        # alternate queues so the 16 MB stream rides two DMA channels
        if jb % 2 == 0:
            nc.sync.dma_start(out=x2b, in_=x2t[jb, :, :, :])
        else:
            nc.gpsimd.dma_start(out=x2b, in_=x2t[jb, :, :, :])
        x2f = loads8.tile([P, NCC, JB], FP8, tag="x2f")
        # fused bf16->fp8 cast + f32-accumulated colsum via accum_out
        for cc in range(NCC):
            if cc < 4:
                nc.vector.tensor_scalar(
                    x2f[:, cc, :], x2b[:, cc, :], 1.0, 0.0,
                    op0=mybir.AluOpType.mult, op1=mybir.AluOpType.add,
                    accum_out=cs_part[:, cc, jb:jb + 1])
            else:
                nc.scalar.activation(
                    out=x2f[:, cc, :], in_=x2b[:, cc, :], func=ACT_COPY,
                    accum_out=cs_part[:, cc, jb:jb + 1])
        # kT block: [512 d, 512 j]; di pairs share a double-bank psum tile,
        # drains stay [128,512] because the bias differs per di
        for dp in range(NDC // 2):
            pk = ps2.tile([P, 2, D], F32, tag="s2")
            for dh in range(2):
                di = 2 * dp + dh
                for cp in range(NCC // 2):
                    nc.tensor.matmul(
                        pk[:, dh, :],
                        lhsT=wk_sb[:, 2 * cp:2 * cp + 2, di * P:(di + 1) * P],
                        rhs=x2f[:, 2 * cp:2 * cp + 2, :],
                        perf_mode=DR, start=(cp == 0),
                        stop=(cp == NCC // 2 - 1))
                nc.vector.tensor_scalar_add(
                    kt[:, di, jb * JB:(jb + 1) * JB], pk[:, dh, :],
                    bk_sb[:, di:di + 1])
        # v block: [512 j, 512 dv]; jl pairs drained as one [128,2,512] op
        for vp in range(JB // P // 2):
            pv = ps2.tile([P, 2, D], F32, tag="s2")
            for vh in range(2):
                jl = 2 * vp + vh
                for cp in range(NCC // 2):
                    nc.tensor.matmul(
                        pv[:, vh, :],
                        lhsT=x2f[:, 2 * cp:2 * cp + 2, jl * P:(jl + 1) * P],
                        rhs=wv_sb[:, 2 * cp:2 * cp + 2, :],
                        perf_mode=DR, start=(cp == 0),
                        stop=(cp == NCC // 2 - 1))
            jj = jb * (JB // P) + 2 * vp
            nc.scalar.activation(out=vv[:, jj:jj + 2, 0:D - 1],
                                 in_=pv[:, :, 0:D - 1], func=ACT_COPY)

    # wv32 is only needed now (cv); its load rides after the stream dispatches
    nc.gpsimd.dma_start(out=wv32_sb, in_=wvt32[:, :, :])

    # ---- colsum_v (fp32) + broadcast helpers ----
    cs = cspool.tile([P, NCC], F32, tag="cs")
    nc.vector.reduce_sum(out=cs, in_=cs_part, axis=mybir.AxisListType.X)
    ps_cv = ps2.tile([P, 2, D], F32, tag="s2")
    for ci in range(NCC):
        nc.tensor.matmul(ps_cv[0:1, 0, :], lhsT=cs[:, ci:ci + 1],
                         rhs=wv32_sb[:, ci, :],
                         start=(ci == 0), stop=(ci == NCC - 1))
    cv1 = cspool.tile([1, D], F32, tag="cv1")
    nc.scalar.activation(out=cv1, in_=ps_cv[0:1, 0, :], func=ACT_COPY)
    cvd1 = cspool.tile([1, D], F32, tag="cvd1")
    nc.vector.tensor_scalar_mul(cvd1, cv1, float(INV_NM1))
    nc.vector.tensor_add(cvd1, cvd1, bv1)
    ps_b = ps2.tile([P, 2, D], F32, tag="s2")
    nc.tensor.matmul(ps_b[:, 0, :], lhsT=ones_row, rhs=cv1,
                     start=True, stop=True)
    nc.tensor.matmul(ps_b[:, 1, :], lhsT=ones_row, rhs=cvd1,
                     start=True, stop=True)
    cv_b = cspool.tile([P, D], F32, tag="cvb")
    nc.vector.tensor_copy(out=cv_b, in_=ps_b[:, 0, :])
    cvd_b = cspool.tile([P, D], F32, tag="cvdb")
    nc.vector.tensor_copy(out=cvd_b, in_=ps_b[:, 1, :])

    # ---- main attention loop, one i-half (512 rows) at a time ----
    for ih in range(2):
        ps_av = [ps2.tile([P, 2, D], F32, tag="s2", name=f"av{ap}_{ih}")
                 for ap in range(2)]
        t2f = tfull.tile([P, NJC, D], FP8, tag="t2f")
        for jp in range(NJC // 2):
            ps_s = ps2.tile([P, 2, D], F32, tag="s2")
            for sh in range(2):
                jc = 2 * jp + sh
                for qp in range(2):
                    nc.tensor.matmul(
                        ps_s[:, sh, :],
                        lhsT=kt[:, 2 * qp:2 * qp + 2, jc * P:(jc + 1) * P],
                        rhs=qt[:, 2 * qp:2 * qp + 2, ih * D:(ih + 1) * D],
                        perf_mode=DR, start=(qp == 0), stop=(qp == 1))
            nc.scalar.activation(out=t2f[:, 2 * jp:2 * jp + 2, :], in_=ps_s,
                                 func=ACT_TANH)
            first = (jp == 0)
            last = (jp == NJC // 2 - 1)
            for si in range(4):
                nc.tensor.matmul(
                    ps_av[si // 2][:, si % 2, :],
                    lhsT=t2f[:, 2 * jp:2 * jp + 2, si * P:(si + 1) * P],
                    rhs=vv[:, 2 * jp:2 * jp + 2, :],
                    perf_mode=DR, start=first, stop=last)

        # ---- epilogue for this i-half: rowsum_i sits at accumulator col 511
        for si in range(4):
            avs = ps_av[si // 2][:, si % 2, :]
            rinv = epool.tile([P, 1], F32, tag="rinv")
            nc.vector.tensor_scalar(rinv, avs[:, D - 1:D], float(SCALE),
                                    float(N), op0=mybir.AluOpType.mult,
                                    op1=mybir.AluOpType.add)
            nc.vector.reciprocal(rinv, rinv)
            ra = epool.tile([P, 1], F32, tag="ra")    # rinv/(N-1)
            nc.vector.tensor_scalar_mul(ra, rinv, float(INV_NM1))
            rb = epool.tile([P, 1], F32, tag="rb")    # rinv*scale/(N-1)
            nc.vector.tensor_scalar_mul(rb, rinv, float(SCALE * INV_NM1))
            o1 = epool.tile([P, D], F32, tag="o1")
            nc.vector.tensor_scalar_mul(o1, avs, rb)
            o2 = epool.tile([P, D], F32, tag="o2")
            nc.gpsimd.tensor_scalar_mul(o2, cv_b, ra)
            nc.vector.tensor_sub(o1, cvd_b, o1)
            nc.vector.tensor_sub(o1, o1, o2)
            nc.sync.dma_start(
                out=out[ih * D + si * P: ih * D + (si + 1) * P, :], in_=o1)


def build_kernel(repeat: int = 1):
    nc = bacc.Bacc(num_devices=NCORES)

    x1t = nc.declare_dram_parameter("x1t", [P, NCC, S], FP8, isOutput=False)
    x2t = nc.declare_dram_parameter("x2t", [NJB, P, NCC, JB], BF16,
                                    isOutput=False)
    wqt = nc.declare_dram_parameter("wqt", [P, NCC, D], FP8, isOutput=False)
    wkt = nc.declare_dram_parameter("wkt", [P, NCC, D], FP8, isOutput=False)
    wvt = nc.declare_dram_parameter("wvt", [P, NCC, D], FP8, isOutput=False)
    wvt32 = nc.declare_dram_parameter("wvt32", [P, NCC, D], F32, isOutput=False)
    bqt = nc.declare_dram_parameter("bqt", [P, NDC], F32, isOutput=False)
    bkt = nc.declare_dram_parameter("bkt", [P, NDC], F32, isOutput=False)
    bv = nc.declare_dram_parameter("bv", [1, D], F32, isOutput=False)
    out = nc.declare_dram_parameter("out", [S, D], F32, isOutput=True)
    io = (x1t, x2t, wqt, wkt, wvt, wvt32, bqt, bkt, bv, out)

    with tile.TileContext(nc) as tc, ExitStack() as ctx:
        persist = ctx.enter_context(tc.tile_pool(name="persist", bufs=1))
        ones_row = persist.tile([1, P], F32)      # broadcast helper
        nc.vector.memset(ones_row, 1.0)
        persist_tiles = (ones_row,)

        wpool = ctx.enter_context(tc.tile_pool(name="weights", bufs=1))
        kvpool = ctx.enter_context(tc.tile_pool(name="kv", bufs=1))
        tfull = ctx.enter_context(tc.tile_pool(name="tfull", bufs=1))
        loads8 = ctx.enter_context(tc.tile_pool(name="loads8", bufs=4))
        loadsbf = ctx.enter_context(tc.tile_pool(name="loadsbf", bufs=4))
        epool = ctx.enter_context(tc.tile_pool(name="epool", bufs=2))
        cspool = ctx.enter_context(tc.tile_pool(name="cspool", bufs=1))
        ps2 = ctx.enter_context(tc.tile_pool(name="ps2", bufs=4, space="PSUM"))
        pools = (wpool, kvpool, tfull, loads8, loadsbf, epool, cspool, ps2,
                 None)

        if repeat == 1:
            emit_body(nc, tc, io, persist_tiles, pools)
        else:
            with tc.For_i(0, repeat, 1,
                          staggered_reset=True,
                          hint_engines=(mybir.EngineType.PE,
                                        mybir.EngineType.Activation,
                                        mybir.EngineType.DVE,
                                        mybir.EngineType.SP,
                                        mybir.EngineType.Pool)):
                emit_body(nc, tc, io, persist_tiles, pools)

    if not nc.is_finalized():
        nc.finalize()
    return nc


_NC_CACHE = {}


def _get_nc(repeat: int = 1):
    if repeat not in _NC_CACHE:
        _NC_CACHE[repeat] = build_kernel(repeat)
    return _NC_CACHE[repeat]


def make_in_maps(x_1, x_2, Wq, bq, Wk, bk, Wv, bv):
    f8 = mybir.dt.np(FP8)
    bf = mybir.dt.np(BF16)

    def chunked_t(a, dtype):
        # [rows, cin] -> transposed, feature-chunked [128, cin//128, rows]
        a = np.asarray(a, np.float32)
        cin, rows = a.shape[1], a.shape[0]
        return np.ascontiguousarray(
            a.T.reshape(cin // P, P, rows).transpose(1, 0, 2)).astype(dtype)

    def blocked(a):
        # [128, 8, N] -> j-blocked [16, 128, 8, 512] (contiguous per block)
        return np.ascontiguousarray(
            a.reshape(P, NCC, NJB, JB).transpose(2, 0, 1, 3))

    x1t = chunked_t(x_1, f8)                      # [128, 8, 8192]
    shared = {
        "x2t": blocked(chunked_t(x_2, bf)),
        "wqt": chunked_t(np.asarray(Wq), f8),     # [128, 8, 512]
        "wkt": chunked_t(np.asarray(Wk), f8),
        "wvt": chunked_t(np.asarray(Wv), f8),
        "wvt32": chunked_t(np.asarray(Wv), np.float32),
        "bqt": np.ascontiguousarray(
            np.asarray(bq, np.float32).reshape(NDC, P).T),
        "bkt": np.ascontiguousarray(
            np.asarray(bk, np.float32).reshape(NDC, P).T),
        "bv": np.asarray(bv, np.float32).reshape(1, D).copy(),
    }
    return [
        {"x1t": np.ascontiguousarray(x1t[:, :, c * S:(c + 1) * S]), **shared}
        for c in range(NCORES)
    ]


def kernel(x_1, x_2, Wq, bq, Wk, bk, Wv, bv):
    nc = _get_nc(1)
    in_maps = make_in_maps(x_1, x_2, Wq, bq, Wk, bk, Wv, bv)
    res = run_bass_kernel_spmd(nc, in_maps, core_ids=list(range(NCORES)))
    return np.concatenate([res.results[c]["out"] for c in range(NCORES)], axis=0)
